# revision 1
# baseline (speedup 1.0000x reference)
"""Trainium2 Bass kernel for nn_DGC_Attention (global-context attention block).

Math (per batch b):
    cm[s]   = sum_c x[b,c,s] * wm[c]            (+ bm, which cancels in softmax)
    mask[s] = softmax(cm)[s] + 1/S              (uniform part: softmax of zeros)
    ctx[c]  = sum_s x[b,c,s] * mask[s]
    t       = relu(LN(ctx @ w1.T + b1) * ln_g + ln_b)
    out     = t @ w2.T + b2                     -> [B, C, 1, 1]

Sharding: pure data parallel, batch dim (16) over 8 cores, 2 batches/core.

Per-core pipeline (single HBM pass over x):
    - x streamed as 8 chunks [128, 4096] per batch into SBUF (f32r tiles).
    - PE computes cm via f32r matmuls (wm chunk as stationary) accumulating
      into one [1, 4096] PSUM tile (8 banks).
    - ACT: one Exp over the bank-spanning PSUM row, accum_out = Z.
    - DVE: zinv = 1/Z ; mask = e*zinv + 1/S  (tensor_scalar, [1,4096]).
    - GPSIMD: partition_broadcast mask -> maskB [128, 4096].
    - DVE: fused multiply+reduce (scalar_tensor_tensor accum_out) per chunk
      -> ctx column [128, 1].  Chunk tiles freed here; next batch's DMA reuses.
    - MLP on PE/DVE in transposed layout ([64, 2], r on partitions): LN via
      gpsimd partition_all_reduce; w1/w2 fed pre-transposed from the host.
"""
import numpy as np

B_PER_CORE = 2
N_CORES = 8
C = 1024
S = 4096
R = 64
NCHUNK = C // 128
LN_EPS = 1e-5

_CACHE = {}


def _build():
    import concourse.bass as bass
    import concourse.tile as tile
    from concourse import bacc, mybir

    f32 = mybir.dt.float32
    f32r = mybir.dt.float32r
    bf16 = mybir.dt.bfloat16
    AF = mybir.ActivationFunctionType
    ALU = mybir.AluOpType

    nc = bacc.Bacc("TRN2", target_bir_lowering=False, debug=False, num_devices=N_CORES)

    x_d = nc.dram_tensor("x", [B_PER_CORE, C, S], f32, kind="ExternalInput").ap()
    wmT_d = nc.dram_tensor("wmT", [128, NCHUNK], f32, kind="ExternalInput").ap()
    w1t_d = nc.dram_tensor("w1t", [128, NCHUNK * R], f32, kind="ExternalInput").ap()
    w2t_d = nc.dram_tensor("w2t", [R, C], f32, kind="ExternalInput").ap()
    b1_d = nc.dram_tensor("b1c", [R, 1], f32, kind="ExternalInput").ap()
    lng_d = nc.dram_tensor("lngc", [R, 1], f32, kind="ExternalInput").ap()
    lnb_d = nc.dram_tensor("lnbc", [R, 1], f32, kind="ExternalInput").ap()
    b2_d = nc.dram_tensor("b2r", [B_PER_CORE, C], f32, kind="ExternalInput").ap()
    out_d = nc.dram_tensor("out", [B_PER_CORE, C], f32, kind="ExternalOutput").ap()

    with tile.TileContext(nc) as tc:
        with (
            tc.tile_pool(name="xp", bufs=NCHUNK) as xp,
            tc.tile_pool(name="cp", bufs=1) as cp,
            tc.tile_pool(name="wp", bufs=1) as wp,
            tc.tile_pool(name="ps", bufs=1, space="PSUM") as ps,
        ):
            wmT = cp.tile([128, NCHUNK], f32r, tag="wmT")
            nc.sync.dma_start(wmT[:], wmT_d.bitcast(f32r))
            w1t = cp.tile([128, NCHUNK * R], f32, tag="w1t")
            nc.sync.dma_start(w1t[:], w1t_d)
            w2t = cp.tile([R, C], f32, tag="w2t")
            nc.sync.dma_start(w2t[:], w2t_d)
            b1c = cp.tile([R, 1], f32, tag="b1c")
            nc.sync.dma_start(b1c[:], b1_d)
            lngc = cp.tile([R, 1], f32, tag="lngc")
            nc.sync.dma_start(lngc[:], lng_d)
            lnbc = cp.tile([R, 1], f32, tag="lnbc")
            nc.sync.dma_start(lnbc[:], lnb_d)
            b2r = cp.tile([B_PER_CORE, C], f32, tag="b2r")
            nc.sync.dma_start(b2r[:], b2_d)

            ctx = wp.tile([128, 2 * NCHUNK], f32, tag="ctx")

            # PE warm-up / wait absorber (fp32 so any-N is legal)
            dum = ps.tile([1, 1], f32, tag="big")
            nc.tensor.matmul(
                dum[:], wmT[:, :1].bitcast(f32), wmT[:, :1].bitcast(f32),
                start=True, stop=True,
            )

            for b in range(B_PER_CORE):
                xt = []
                for k in range(NCHUNK):
                    t = xp.tile([128, S], f32r, tag="x")
                    nc.sync.dma_start(
                        t[:], x_d[b, 128 * k : 128 * (k + 1), :].bitcast(f32r)
                    )
                    xt.append(t)

                big = ps.tile([1, S], f32, tag="big")
                for k in range(NCHUNK):
                    for j in range(S // 512):
                        nc.tensor.matmul(
                            big[:, 512 * j : 512 * (j + 1)],
                            wmT[:, k : k + 1],
                            xt[k][:, 512 * j : 512 * (j + 1)],
                            start=(k == 0),
                            stop=(k == NCHUNK - 1),
                        )

                e = wp.tile([1, S], f32, tag="e")
                z = wp.tile([1, 1], f32, tag="z")
                nc.scalar.activation(e[:], big[:], AF.Exp, accum_out=z[:])
                zinv = wp.tile([1, 1], f32, tag="zinv")
                nc.vector.reciprocal(zinv[:], z[:])
                mask = wp.tile([1, S], f32, tag="mask")
                nc.vector.tensor_scalar(
                    out=mask[:], in0=e[:], scalar1=zinv[:], scalar2=1.0 / S,
                    op0=ALU.mult, op1=ALU.add,
                )
                maskB = wp.tile([128, S], f32, tag="maskB")
                nc.gpsimd.partition_broadcast(maskB[:], mask[:])

                scratch = wp.tile([128, S], bf16, tag="scratch")
                for k in range(NCHUNK):
                    nc.vector.scalar_tensor_tensor(
                        out=scratch[:],
                        in0=xt[k][:].bitcast(f32),
                        scalar=1.0,
                        in1=maskB[:],
                        op0=ALU.mult,
                        op1=ALU.mult,
                        accum_out=ctx[:, 2 * k + b : 2 * k + b + 1],
                    )

            # ---- MLP tail (both batches together) ----
            from concourse import bass_isa

            tps = ps.tile([R, B_PER_CORE], f32, tag="big")
            for k in range(NCHUNK):
                nc.tensor.matmul(
                    tps[:],
                    w1t[:, R * k : R * (k + 1)],
                    ctx[:, 2 * k : 2 * k + 2],
                    start=(k == 0),
                    stop=(k == NCHUNK - 1),
                )
            t_sb = wp.tile([R, B_PER_CORE], f32, tag="t_sb")
            nc.vector.tensor_scalar(
                out=t_sb[:], in0=tps[:], scalar1=b1c[:], scalar2=None, op0=ALU.add
            )
            # LayerNorm over r (partition dim) via gpsimd all-reduce
            s1 = wp.tile([R, B_PER_CORE], f32, tag="s1")
            nc.gpsimd.partition_all_reduce(s1[:], t_sb[:], R, bass_isa.ReduceOp.add)
            mu = wp.tile([R, B_PER_CORE], f32, tag="mu")
            nc.vector.tensor_scalar_mul(mu[:], s1[:], 1.0 / R)
            tctr = wp.tile([R, B_PER_CORE], f32, tag="tctr")
            nc.vector.tensor_sub(tctr[:], t_sb[:], mu[:])
            sq = wp.tile([R, B_PER_CORE], f32, tag="sq")
            nc.vector.tensor_mul(sq[:], tctr[:], tctr[:])
            ss = wp.tile([R, B_PER_CORE], f32, tag="ss")
            nc.gpsimd.partition_all_reduce(ss[:], sq[:], R, bass_isa.ReduceOp.add)
            var = wp.tile([R, B_PER_CORE], f32, tag="var")
            nc.vector.tensor_scalar(
                out=var[:], in0=ss[:], scalar1=1.0 / R, scalar2=LN_EPS,
                op0=ALU.mult, op1=ALU.add,
            )
            std = wp.tile([R, B_PER_CORE], f32, tag="std")
            nc.scalar.sqrt(std[:], var[:])
            rstd = wp.tile([R, B_PER_CORE], f32, tag="rstd")
            nc.vector.reciprocal(rstd[:], std[:])
            tn = wp.tile([R, B_PER_CORE], f32, tag="tn")
            nc.vector.tensor_mul(tn[:], tctr[:], rstd[:])
            tg = wp.tile([R, B_PER_CORE], f32, tag="tg")
            nc.vector.tensor_scalar(
                out=tg[:], in0=tn[:], scalar1=lngc[:], scalar2=lnbc[:],
                op0=ALU.mult, op1=ALU.add,
            )
            tr = wp.tile([R, B_PER_CORE], f32, tag="tr")
            nc.vector.tensor_scalar_max(tr[:], tg[:], 0.0)

            ops_ = ps.tile([B_PER_CORE, C], f32, tag="big")
            for h in range(C // 512):
                nc.tensor.matmul(
                    ops_[:, 512 * h : 512 * (h + 1)],
                    tr[:],
                    w2t[:, 512 * h : 512 * (h + 1)],
                    start=True,
                    stop=True,
                )
            out_sb = wp.tile([B_PER_CORE, C], f32, tag="out_sb")
            nc.vector.tensor_add(out_sb[:], ops_[:], b2r[:])
            nc.sync.dma_start(out_d[:], out_sb[:])

    nc.compile()
    return nc


def _prep_inputs(x, wm, w1, b1, ln_g, ln_b, w2, b2):
    x = np.ascontiguousarray(x, dtype=np.float32).reshape(16, C, S)
    wmT = np.ascontiguousarray(wm.astype(np.float32).reshape(NCHUNK, 128).T)
    # w1t[p, 64k+r] = w1[r, 128k+p]
    w1t = np.ascontiguousarray(
        w1.astype(np.float32).reshape(R, NCHUNK, 128).transpose(2, 1, 0).reshape(128, NCHUNK * R)
    )
    w2t = np.ascontiguousarray(w2.astype(np.float32).T)
    b1c = np.ascontiguousarray(b1.astype(np.float32).reshape(R, 1))
    lngc = np.ascontiguousarray(ln_g.astype(np.float32).reshape(R, 1))
    lnbc = np.ascontiguousarray(ln_b.astype(np.float32).reshape(R, 1))
    b2r = np.ascontiguousarray(
        np.broadcast_to(b2.astype(np.float32)[None, :], (B_PER_CORE, C))
    )
    in_maps = []
    for c in range(N_CORES):
        in_maps.append(
            {
                "x": x[B_PER_CORE * c : B_PER_CORE * (c + 1)],
                "wmT": wmT,
                "w1t": w1t,
                "w2t": w2t,
                "b1c": b1c,
                "lngc": lngc,
                "lnbc": lnbc,
                "b2r": b2r,
            }
        )
    return in_maps


def _run(inputs, trace=False, trace_kwargs=None):
    from concourse.bass_utils import run_bass_kernel_spmd

    if "nc" not in _CACHE:
        _CACHE["nc"] = _build()
    nc = _CACHE["nc"]
    in_maps = _prep_inputs(
        inputs["x"], inputs["wm"], inputs["w1"], inputs["b1"],
        inputs["ln_g"], inputs["ln_b"], inputs["w2"], inputs["b2"],
    )
    br = run_bass_kernel_spmd(
        nc, in_maps, list(range(N_CORES)), trace=trace,
        trace_kwargs=trace_kwargs or {},
    )
    out = np.concatenate([np.asarray(r["out"]) for r in br.results], axis=0)
    return out.reshape(16, C, 1, 1).astype(np.float32), br


def kernel(x, wm, bm, w1, b1, ln_g, ln_b, w2, b2):
    inputs = dict(x=x, wm=wm, bm=bm, w1=w1, b1=b1, ln_g=ln_g, ln_b=ln_b, w2=w2, b2=b2)
    out, _ = _run({k: np.asarray(v) for k, v in inputs.items()})
    return out
